# revision 14
# baseline (speedup 1.0000x reference)
"""Trainium2 Bass kernel for nn_Contexture (class-center attention + 1x1 conv + BN + ReLU).

kernel(**inputs): FULL inputs (np arrays per setup_inputs) -> FULL output
[8, 6, 10, 128, 128] f32. Data-parallel over batch across 8 NeuronCores;
BN batch stats via AllReduce.

Math (verified vs reference in numpy, rel err ~8e-7):
  wT[s,k]  = exp(p_seg[k,s]) / sum_k' exp(p_seg[k',s])      (att^T; no max-sub needed)
  [A; key] = [Wp1; 0; Wk] @ fea        (one 74-row matmul; A rows 0:60, key rows 64:74)
  GT[s,:]  = ([Wq; Wp2] @ fea)^T       (per 128-col block, fea tile stationary)
  wnumT    = wT^T @ GT                 == ([Wq@num; Wp2@num])^T where num = fea @ wT
  den[k]   = sum_s wT[s,k]; wnum_s = wnumT * (1/den)  (p_center = num/den folded in)
  query    = wnum_s[:, :10]^T + bq;  BT = wnum_s[:, 10:]
  PT       = exp(key^T @ query)        ([s, k] layout, 128-lane friendly)
  PnT      = PT / sum_k PT             (free-axis reduce + bcast mult)
  Pp       = PnT transposed back to [7*j'+k, g', 512] via strided PE transposes
  ypre     = A + BT^T @ Pp;  BN(batch stats all-reduced);  ReLU
"""
import sys
import numpy as np

try:
    import concourse.bacc as bacc  # noqa: F401
except Exception:  # pragma: no cover
    for p in ("/opt/trn_rl_repo", "/root/.axon_site/_ro/trn_rl_repo"):
        if p not in sys.path:
            sys.path.insert(0, p)
    import concourse.bacc as bacc

import concourse.bass as bass  # noqa: F401
import concourse.tile as tile
from concourse import mybir
from concourse.bass_utils import run_bass_kernel_spmd

F32 = mybir.dt.float32
F32R = mybir.dt.float32r
BF16 = mybir.dt.bfloat16
AF = mybir.ActivationFunctionType
ALU = mybir.AluOpType

NCORES = 8
C, S, K, HID, OC = 256, 16384, 7, 10, 60
MA = 74                # [A(60) | pad(4) | key(10)]
CHUNK = 1024           # fea columns per DMA chunk
NCHUNK = S // CHUNK    # 16
NBLK = S // 128        # 128 col-blocks
NSL = S // 512         # 32 slices
NG = S // 2048         # 8 pseg pack groups
BN_EPS = 1e-5
MMDT = F32R       # dtype for big-N matmuls (f32r: 1 cyc/row at N>=256)
import os as _os
_SKIP = set(_os.environ.get("KM_SKIP", "").split(","))       # dtype for big-N matmuls (f32r: 1 cyc/row at N>=256)


def _emit_body(tc, nc, d, with_collective):
    ctx = d["ctx"]
    singles = ctx.enter_context(tc.tile_pool(name="singles", bufs=1))
    fea_pool = ctx.enter_context(tc.tile_pool(name="fea", bufs=3))
    gt_pool = ctx.enter_context(tc.tile_pool(name="gt", bufs=3))
    out_pool = ctx.enter_context(tc.tile_pool(name="outp", bufs=2))
    small = ctx.enter_context(tc.tile_pool(name="small", bufs=2))
    psMM = ctx.enter_context(tc.tile_pool(name="psMM", bufs=3, space="PSUM"))
    psGT = ctx.enter_context(tc.tile_pool(name="psGT", bufs=2, space="PSUM"))
    psSm = ctx.enter_context(tc.tile_pool(name="psSm", bufs=2, space="PSUM"))
    psAcc = ctx.enter_context(tc.tile_pool(name="psAcc", bufs=1, space="PSUM"))

    # ---- resident constants ----
    w1t = singles.tile([128, 2, MA], F32R)       # [Wp1 | 0 | Wk]^T halves
    nc.sync.dma_start(out=w1t[:], in_=d["w1t"][:].bitcast(F32R))
    w2t = singles.tile([128, 2, 70], F32R)       # [Wq | Wp2]^T halves
    nc.sync.dma_start(out=w2t[:], in_=d["w2t"][:].bitcast(F32R))
    bias74 = singles.tile([MA, 1], F32)
    nc.sync.dma_start(out=bias74[:], in_=d["bias74"][:])
    bq74 = singles.tile([MA, 1], F32)
    nc.sync.dma_start(out=bq74[:], in_=d["bq74"][:])
    gam = singles.tile([OC, 1], F32)
    nc.sync.dma_start(out=gam[:], in_=d["gamma"][:])
    bet = singles.tile([OC, 1], F32)
    nc.sync.dma_start(out=bet[:], in_=d["beta"][:])
    ones128 = singles.tile([128, 1], F32)
    nc.sync.dma_start(out=ones128[:], in_=d["ones128"][:])
    ident = singles.tile([128, 128], F32)
    nc.sync.dma_start(out=ident[:], in_=d["ident"][:])
    eps_t = singles.tile([OC, 1], F32)
    nc.vector.memset(eps_t[:], BN_EPS)

    # ---- stage 0: p_seg -> wT ----
    pseg_pack = singles.tile([112, NG, 128], F32)
    pseg_r = d["pseg"].rearrange("k (g j i) -> k g j i", g=NG, j=16, i=128)
    for j in range(16):
        nc.sync.dma_start(out=pseg_pack[7 * j:7 * j + 7, :, :], in_=pseg_r[:, :, j, :])
    eT = singles.tile([128, NG, 112], F32)
    for g in range(NG):
        pt = psSm.tile([128, 112], F32, tag="sm")
        nc.tensor.transpose(pt[:], pseg_pack[:, g, :], ident[0:112, 0:112])
        nc.scalar.activation(out=eT[:, g, :], in_=pt[:], func=AF.Exp)
    eT4 = eT[:].rearrange("p g (j k) -> p g j k", k=7)
    zsum = small.tile([128, NG, 16], F32)
    nc.vector.tensor_reduce(zsum[:], eT4, axis=mybir.AxisListType.X, op=ALU.add)
    zinv = small.tile([128, NG, 16], F32)
    nc.vector.reciprocal(zinv[:], zsum[:])
    wT = singles.tile([128, NG, 112], F32)
    wT4 = wT[:].rearrange("p g (j k) -> p g j k", k=7)
    nc.vector.tensor_tensor(wT4, eT4, zinv[:, :, :, None].to_broadcast((128, NG, 16, 7)), op=ALU.mult)
    wT_bf = singles.tile([128, NG, 112], BF16)
    nc.vector.tensor_copy(wT_bf[:], wT[:])
    wT4_bf = wT_bf[:].rearrange("p g (j k) -> p g j k", k=7)
    wT_part = small.tile([128, 7], F32)
    nc.vector.tensor_reduce(wT_part[:], wT4.transpose([0, 3, 1, 2]), axis=mybir.AxisListType.XY, op=ALU.add)
    ps_den = psSm.tile([7, 1], F32, tag="sm")
    nc.tensor.matmul(ps_den[:], wT_part[:], ones128[:], start=True, stop=True)
    denInv = small.tile([7, 1], F32)
    nc.vector.reciprocal(denInv[:], ps_den[:])

    # ---- phase 1: stream fea; [A|pad|key] + GT + wnumT accumulation ----
    keyA = singles.tile([MA, S], F32)           # rows 0:60 A, 64:74 key
    ps_wnum = psAcc.tile([7, 70], F32, tag="wnum")
    for ci in range(NCHUNK):
        fch = fea_pool.tile([128, 2, CHUNK], F32R, tag="fea")
        nc.sync.dma_start(out=fch[:, 0, :], in_=d["fea"][0:128, ci * CHUNK:(ci + 1) * CHUNK].bitcast(F32R))
        nc.sync.dma_start(out=fch[:, 1, :], in_=d["fea"][128:256, ci * CHUNK:(ci + 1) * CHUNK].bitcast(F32R))
        for t in range(CHUNK // 512) if "A" not in _SKIP else []:
            col0 = ci * CHUNK + t * 512
            pa = psMM.tile([MA, 512], F32, tag="mm")
            nc.tensor.matmul(pa[:], w1t[:, 0, :], fch[:, 0, t * 512:(t + 1) * 512], start=True, stop=False)
            nc.tensor.matmul(pa[:], w1t[:, 1, :], fch[:, 1, t * 512:(t + 1) * 512], start=False, stop=True)
            nc.scalar.activation(out=keyA[:, col0:col0 + 512], in_=pa[:], func=AF.Identity, bias=bias74[:])
        for q in range(CHUNK // 512) if "B" not in _SKIP else []:
            pg = psGT.tile([128, 4, 70], F32, tag="gt")
            for u in range(4):
                blk = (q * 4 + u) * 128
                nc.tensor.matmul(pg[:, u, :], fch[:, 0, blk:blk + 128], w2t[:, 0, :], start=True, stop=False)
                nc.tensor.matmul(pg[:, u, :], fch[:, 1, blk:blk + 128], w2t[:, 1, :], start=False, stop=True)
            gt = gt_pool.tile([128, 4, 70], BF16, tag="gt_sb")
            nc.vector.tensor_copy(gt[:], pg[:])
            for u in range(4):
                b = ci * (CHUNK // 128) + q * 4 + u
                g, j = b // 16, b % 16
                nc.tensor.matmul(ps_wnum[:], wT4_bf[:, g, j, :], gt[:, u, :],
                                 start=(b == 0), stop=(b == NBLK - 1))

    # ---- small: query / BT ----
    wnum_s = small.tile([7, 70], F32)
    nc.scalar.activation(out=wnum_s[:], in_=ps_wnum[:], func=AF.Copy, scale=denInv[:])
    # query computed at base 0, then DMA-shifted to partitions 64:74 so the
    # energy matmul lhsT (key rows, base 64) and rhs (query) share a base.
    ps_q = psSm.tile([HID, 7], F32, tag="sm")
    nc.tensor.transpose(ps_q[:], wnum_s[:, 0:HID], ident[0:7, 0:7])
    query0 = small.tile([HID, 7], F32, tag="query0")
    nc.scalar.activation(out=query0[:], in_=ps_q[:], func=AF.Identity, bias=bq74[0:HID, :])
    query = small.tile([MA, 7], F32)
    nc.sync.dma_start(out=query[64:MA, :], in_=query0[:])
    # BT_exp[7j+k, j', o] = BT[k, o] if j == j' else 0 (zero-padded shifts, built via DMA)
    BT_exp = singles.tile([112, 16, OC], F32)
    nc.vector.memset(BT_exp[:], 0.0)
    for j in range(16):
        nc.sync.dma_start(out=BT_exp[7 * j:7 * j + 7, j, :], in_=wnum_s[:, HID:70])
    btx_bf = singles.tile([112, 16, OC], BF16)
    nc.vector.tensor_copy(btx_bf[:], BT_exp[:])

    # ---- phase 2: energyT -> PT -> PnT -> Pp -> ypre ----
    PT = singles.tile([128, NBLK, 7], F32)
    for v in range(NBLK // 4) if "E" not in _SKIP else []:
        pe = psMM.tile([128, 4, 7], F32, tag="mm")
        for u in range(4):
            blk = (v * 4 + u) * 128
            nc.tensor.matmul(pe[:, u, :], keyA[64:MA, blk:blk + 128], query[64:MA, :], start=True, stop=True)
        nc.scalar.activation(out=PT[:, v * 4:v * 4 + 4, :], in_=pe[:], func=AF.Exp)
    z2 = small.tile([128, NBLK], F32, tag="z2")
    nc.vector.tensor_reduce(z2[:], PT[:], axis=mybir.AxisListType.X, op=ALU.add)
    z2inv = small.tile([128, NBLK], F32, tag="z2i")
    nc.vector.reciprocal(z2inv[:], z2[:])
    PnT = singles.tile([128, NBLK, 7], F32)
    nc.vector.tensor_tensor(PnT[:], PT[:], z2inv[:, :, None].to_broadcast((128, NBLK, 7)), op=ALU.mult)
    Pp128 = singles.tile([112, 8, 128], BF16)
    for half in range(2):
        tp = psMM.tile([112, 4, 128], F32, tag="mm")
        for c in range(4):
            m = half * 4 + c
            nc.tensor.transpose(tp[:, c, :], PnT[:, 16 * m:16 * m + 16, :], ident[:])
        nc.vector.tensor_copy(Pp128[:, half * 4:half * 4 + 4, :], tp[:])

    ypre = singles.tile([OC, S], F32)
    stats = small.tile([OC, NSL, 6], F32, tag="stats")
    for t in range(NSL) if "Y" not in _SKIP else []:
        py = psMM.tile([OC, 4, 128], F32, tag="mm")
        for u in range(4):
            b = 4 * t + u
            nc.tensor.matmul(py[:, u, :], btx_bf[:, b % 16, :], Pp128[:, b // 16, :],
                             start=True, stop=True)
        sl = slice(t * 512, (t + 1) * 512)
        nc.vector.tensor_add(ypre[:, sl], py[:].rearrange("p a b -> p (a b)"), keyA[0:OC, sl])
        nc.vector.bn_stats(out=stats[:, t, :], in_=ypre[:, sl])
    mv = small.tile([OC, 2], F32, tag="mv")
    nc.vector.bn_aggr(out=mv[:], in_=stats[:])

    # ---- BN coefficients (cross-core batch stats) ----
    payload = small.tile([OC, 2], F32, tag="payload")
    nc.vector.tensor_copy(payload[:, 0:1], mv[:, 0:1])
    nc.vector.scalar_tensor_tensor(payload[:, 1:2], mv[:, 0:1], mv[:, 0:1], mv[:, 1:2],
                                   op0=ALU.mult, op1=ALU.add)
    gstats = small.tile([OC, 2], F32, tag="gstats")
    if with_collective:
        nc.sync.dma_start(out=d["cc_in"][:], in_=payload[:])
        nc.gpsimd.collective_compute(
            "AllReduce", ALU.add,
            replica_groups=[list(range(NCORES))],
            ins=[d["cc_in"][:]], outs=[d["cc_out"][:]],
        )
        nc.sync.dma_start(out=gstats[:], in_=d["cc_out"][:])
        scale = 1.0 / NCORES
    else:
        nc.vector.tensor_copy(gstats[:], payload[:])
        scale = 1.0
    mu = small.tile([OC, 1], F32, tag="mu")
    nc.vector.tensor_scalar_mul(mu[:], gstats[:, 0:1], scale)
    e2 = small.tile([OC, 1], F32, tag="e2")
    nc.vector.tensor_scalar_mul(e2[:], gstats[:, 1:2], scale)
    var = small.tile([OC, 1], F32, tag="var")
    nc.vector.scalar_tensor_tensor(var[:], mu[:], -1.0, mu[:], op0=ALU.mult, op1=ALU.mult)
    nc.vector.tensor_add(var[:], var[:], e2[:])
    sd = small.tile([OC, 1], F32, tag="sd")
    nc.scalar.activation(out=sd[:], in_=var[:], func=AF.Sqrt, bias=eps_t[:])
    rstd = small.tile([OC, 1], F32, tag="rstd")
    nc.vector.reciprocal(rstd[:], sd[:])
    a_c = small.tile([OC, 1], F32, tag="a_c")
    nc.vector.tensor_mul(a_c[:], gam[:], rstd[:])
    b_c = small.tile([OC, 1], F32, tag="b_c")
    nc.vector.scalar_tensor_tensor(b_c[:], mu[:], -1.0, a_c[:], op0=ALU.mult, op1=ALU.mult)
    nc.vector.tensor_add(b_c[:], b_c[:], bet[:])

    # ---- final: ReLU(a*ypre + b) -> out ----
    for to in range(NSL // 4) if "F" not in _SKIP else []:
        ot = out_pool.tile([OC, 4, 512], F32, tag="out")
        for u in range(4):
            t = to * 4 + u
            nc.scalar.activation(out=ot[:, u, :], in_=ypre[:, t * 512:(t + 1) * 512],
                                 func=AF.Relu, bias=b_c[:], scale=a_c[:])
        nc.sync.dma_start(out=d["out"][:, to * 2048:(to + 1) * 2048], in_=ot[:].rearrange("p a b -> p (a b)"))


def build(loop_R=None, with_collective=True):
    from contextlib import ExitStack
    nc = bacc.Bacc("TRN2", target_bir_lowering=False, debug=False,
                   enable_asserts=False, num_devices=NCORES)
    d = {}
    d["fea"] = nc.dram_tensor("fea", [C, S], F32, kind="ExternalInput").ap()
    d["pseg"] = nc.dram_tensor("pseg", [K, S], F32, kind="ExternalInput").ap()
    d["w1t"] = nc.dram_tensor("w1t", [128, 2, MA], F32, kind="ExternalInput").ap()
    d["w2t"] = nc.dram_tensor("w2t", [128, 2, 70], F32, kind="ExternalInput").ap()
    d["bias74"] = nc.dram_tensor("bias74", [MA, 1], F32, kind="ExternalInput").ap()
    d["bq74"] = nc.dram_tensor("bq74", [MA, 1], F32, kind="ExternalInput").ap()
    d["gamma"] = nc.dram_tensor("gamma", [OC, 1], F32, kind="ExternalInput").ap()
    d["beta"] = nc.dram_tensor("beta", [OC, 1], F32, kind="ExternalInput").ap()
    d["ones128"] = nc.dram_tensor("ones128", [128, 1], F32, kind="ExternalInput").ap()
    d["ident"] = nc.dram_tensor("ident", [128, 128], F32, kind="ExternalInput").ap()
    d["out"] = nc.dram_tensor("out", [OC, S], F32, kind="ExternalOutput").ap()
    if with_collective:
        d["cc_in"] = nc.dram_tensor("cc_in", [OC, 2], F32).ap()
        d["cc_out"] = nc.dram_tensor("cc_out", [OC, 2], F32, addr_space="Shared").ap()

    with tile.TileContext(nc) as tc:
        with ExitStack() as ctx:
            d["ctx"] = ctx
            if loop_R is None:
                _emit_body(tc, nc, d, with_collective)
            else:
                with tc.For_i(0, loop_R, 1):
                    _emit_body(tc, nc, d, with_collective)
    nc.compile()
    return nc


def host_inputs(p_fea, p_seg, Wq, bq, Wk, bk, Wp, gamma, beta):
    f32 = np.float32
    Wq, bq, Wk, bk, Wp = (np.asarray(x, f32) for x in (Wq, bq, Wk, bk, Wp))
    gamma, beta = np.asarray(gamma, f32), np.asarray(beta, f32)
    Wp1, Wp2 = Wp[:, :C], Wp[:, C:]
    # w1 rows: [Wp1(60) | pad(4) | Wk(10)] -> transposed [256, 74]
    w1 = np.zeros((MA, C), f32)
    w1[0:OC] = Wp1
    w1[64:74] = Wk
    w1t = w1.T.reshape(2, 128, MA).transpose(1, 0, 2)
    w2t = np.concatenate([Wq.T, Wp2.T], axis=1).reshape(2, 128, 70).transpose(1, 0, 2)
    bias74 = np.zeros((MA, 1), f32)
    bias74[64:74, 0] = bk
    bq74 = np.zeros((MA, 1), f32)
    bq74[0:10, 0] = bq
    shared = {
        "w1t": np.ascontiguousarray(w1t, f32),
        "w2t": np.ascontiguousarray(w2t, f32),
        "bias74": bias74,
        "bq74": bq74,
        "gamma": np.ascontiguousarray(gamma[:, None], f32),
        "beta": np.ascontiguousarray(beta[:, None], f32),
        "ones128": np.ones((128, 1), f32),
        "ident": np.eye(128, dtype=f32),
    }
    maps = []
    for i in range(NCORES):
        m = dict(shared)
        m["fea"] = np.ascontiguousarray(np.asarray(p_fea[i], f32).reshape(C, S))
        m["pseg"] = np.ascontiguousarray(np.asarray(p_seg[i], f32).reshape(K, S))
        maps.append(m)
    return maps


_nc_cache = {}


def kernel(p_fea, p_seg, Wq, bq, Wk, bk, Wp, gamma, beta):
    if "nc" not in _nc_cache:
        _nc_cache["nc"] = build()
    nc = _nc_cache["nc"]
    maps = host_inputs(p_fea, p_seg, Wq, bq, Wk, bk, Wp, gamma, beta)
    res = run_bass_kernel_spmd(nc, maps, list(range(NCORES)))
    out = np.stack([res.results[i]["out"] for i in range(NCORES)])
    return out.reshape(NCORES, K - 1, HID, 128, 128)
